# revision 16
# baseline (speedup 1.0000x reference)
"""Mistral flash-attention (paged KV, GQA, sliding window) on 8 TRN2 cores.

Tensor-parallel over heads: core m owns kv-head m and q-heads 4m..4m+3,
wq/wk/wv column-sharded, wo column-sharded; the attention output is
AllGathered in fp16 per 128-token block (pipelined with attention), then
each core computes its 512 output columns of o @ wo.

v4 layout: everything on-chip is fp16.  1/sqrt(D) is folded into wq on
the host.  Attention iterates per (batch, 128-query tile): the 1024-wide
sliding window needs 9 key tiles, and the boundary masks collapse to two
static 128x128 triangles.  The softmax denominator is computed by a DVE
tree-sum of the 9 expS tiles followed by a single all-ones matmul that
both reduces over keys and broadcasts the result to 128 partitions; the
reciprocal uses the fast fp32 custom-DVE op.  o_proj is interleaved with
the tail of attention: each 128-token block's 32 matmuls are emitted as
soon as its AllGather output can have landed, so the PE never drains at
the end.  The ocb gather tiles share the expS pool (SBUF stays under
budget with hid resident) and o_proj PSUM shares the oTp tag.  All
AG-gated DMA triggers live on the gpsimd queue so a stalled guard can
never block the exp/weight streams.  Input DMAs are ordered so the
hidden-state tiles stream first and the kv pass starts ~9us in.
"""
import os
import sys
import math
import numpy as np

import concourse.bacc as bacc
import concourse.tile as tile
from concourse import mybir
from concourse.bass_utils import run_bass_kernel_spmd

# ---- problem constants (hardcoded per contest rules) ----
HID = 4096; H = 32; KVH = 8; D = 128
B = 4; Q = 256; KV = 2048; HIST = KV - Q
BS = 64; NB = KV // BS; NBLOCKS = 160
WINDOW = 1024; THETA = 10000.0
T = B * Q                      # 1024 tokens
M = 8                          # cores
HPC = H // M                   # 4 q-heads per core
SCALE = 1.0 / math.sqrt(D)

# windowed cache key range: slots (HIST-WINDOW, HIST) come from the cache,
# slots [HIST, KV) are the new tokens computed on-chip.
K0 = HIST - WINDOW             # 768, first cache slot
NCBLK = (HIST - K0) // BS      # 16 cache blocks per seq
CKEYS = NCBLK * BS             # 1024 cache keys per seq
NKT = 9                        # key tiles of 128 per (batch, q-tile)

F32 = mybir.dt.float32
FP16 = mybir.dt.float16

_CACHE = {}


def _build():
    from contextlib import ExitStack
    nc = bacc.Bacc("TRN2", target_bir_lowering=False, debug=False,
                   enable_asserts=False, num_devices=M)

    dt_in = nc.dram_tensor
    hidT = dt_in("hidT", [HID, T], FP16, kind="ExternalInput").ap()
    # wcat[p, c] = [128, 512]: chunk-pair c of the two feature blocks of
    # pass p (kv / q01 / q23)
    wcat = dt_in("wcat", [3, 16, 128, 512], FP16, kind="ExternalInput").ap()
    wo = dt_in("wo", [HID, 512], FP16, kind="ExternalInput").ap()
    kcT = dt_in("kcT", [B, 128, CKEYS], FP16, kind="ExternalInput").ap()
    vc = dt_in("vc", [B, 128, CKEYS], FP16, kind="ExternalInput").ap()
    cosT = dt_in("cosT", [128, T], F32, kind="ExternalInput").ap()
    sinTs = dt_in("sinTs", [128, T], F32, kind="ExternalInput").ap()
    masks = dt_in("masks", [2, 128, 512], FP16, kind="ExternalInput").ap()
    ident = dt_in("ident", [128, 128], FP16, kind="ExternalInput").ap()
    ones2d = dt_in("ones2d", [128, 128], FP16, kind="ExternalInput").ap()
    outp = dt_in("out", [T, 512], F32, kind="ExternalOutput").ap()

    ag_in = [nc.dram_tensor(f"ag_in{s}", [512, 128], FP16).ap() for s in range(8)]
    ag_out = [nc.dram_tensor(f"ag_out{s}", [H * D, 128], FP16,
                             addr_space="Shared").ap() for s in range(8)]

    with tile.TileContext(nc) as tc, ExitStack() as top:
        persist = top.enter_context(tc.tile_pool(name="persist", bufs=1))

        qT = persist.tile([128, HPC * T], FP16, tag="qT")     # (head, token)
        kT = persist.tile([128, T], FP16, tag="kT")
        vnat = persist.tile([128, 8 * 128], FP16, tag="vnat")  # 8 token-tiles
        oT = persist.tile([128, HPC * T], FP16, tag="oT")
        ones_sb = persist.tile([128, 128], FP16, tag="ones2d")
        id_sb = persist.tile([128, 128], FP16, tag="ident")
        mask_sb = persist.tile([128, 2 * 512], FP16, tag="mask")
        wo_sb = persist.tile([128, 32 * 512], FP16, tag="wo_sb")
        kc_sb = persist.tile([128, B * CKEYS], FP16, tag="kc")
        vc_sb = persist.tile([128, B * CKEYS], FP16, tag="vc")

        # tiny warm-up AllGather so the first real AG doesn't pay
        # first-collective overhead; dram->dram copy so nothing gates on SBUF
        warm_in = nc.dram_tensor("warm_in", [1, 128], FP16).ap()
        warm_out = nc.dram_tensor("warm_out", [M, 128], FP16,
                                  addr_space="Shared").ap()
        nc.gpsimd.dma_start(warm_in[:, :], ident[0:1, 0:128])
        nc.gpsimd.collective_compute(
            "AllGather", mybir.AluOpType.bypass,
            replica_groups=[list(range(M))],
            ins=[warm_in.opt()], outs=[warm_out.opt()])

        qT4 = qT[:].rearrange("p (h t) -> p h t", h=HPC)
        oT4 = oT[:].rearrange("p (h t) -> p h t", h=HPC)

        with tc.tile_pool(name="s1", bufs=1) as s1, \
             tc.tile_pool(name="wstream", bufs=10) as ws, \
             tc.tile_pool(name="ropetmp", bufs=2) as rt, \
             tc.tile_pool(name="es", bufs=3) as es, \
             tc.tile_pool(name="tree", bufs=2) as trp, \
             tc.tile_pool(name="rbpool", bufs=2) as rbp, \
             tc.tile_pool(name="s3o", bufs=2) as s3o:

            # hidden-state tiles first on both DGE queues so the kv pass can
            # start as soon as chunk 0 lands
            # hid chunk-pairs + kv weights interleaved so chunk c's operands
            # land in consumption order: hid pairs alternate queues, w0 pairs
            # alternate the other way
            hidb = s1.tile([128, 32 * T], FP16, tag="hidb")

            def hid(c, th):
                return hidb[:, T * c + 512 * th:T * c + 512 * (th + 1)]
            w0tiles = []
            for k in range(16):
                e1, e2 = (nc.scalar, nc.sync) if k % 2 == 0 else (nc.sync, nc.scalar)
                e1.dma_start(
                    hidb[:, 2 * T * k:2 * T * (k + 1)]
                    .rearrange("p (c t) -> p c t", c=2),
                    hidT[256 * k:256 * (k + 1), :]
                    .rearrange("(c p) t -> p c t", c=2))
                wt = ws.tile([128, 512], FP16, tag="w", name=f"w0_{k}")
                e2.dma_start(wt[:], wcat[0, k])
                w0tiles.append(wt)
            # small constants on the gpsimd queue (idle early)
            nc.gpsimd.dma_start(id_sb[:], ident[:, :])
            nc.gpsimd.dma_start(ones_sb[:], ones2d[:, :])
            for i in range(2):
                nc.gpsimd.dma_start(mask_sb[:, 512 * i:512 * (i + 1)], masks[i])
            # rope factors + kv cache behind the hid stream on scalar
            cos_sb = s1.tile([128, T], F32, tag="cos")
            sin_sb = s1.tile([128, T], F32, tag="sin")
            nc.scalar.dma_start(cos_sb[:], cosT[:, :])
            nc.scalar.dma_start(sin_sb[:], sinTs[:, :])
            vT = s1.tile([128, T], FP16, tag="vT")
            for b in range(B):
                nc.scalar.dma_start(kc_sb[:, CKEYS * b:CKEYS * (b + 1)], kcT[b])
                nc.scalar.dma_start(vc_sb[:, CKEYS * b:CKEYS * (b + 1)], vc[b])

            def rope(ps, sl, dest, tag):
                t1 = rt.tile([128, 512], F32, tag="t1", name=f"t1{tag}")
                t2 = rt.tile([128, 512], F32, tag="t2", name=f"t2{tag}")
                nc.vector.tensor_mul(t1[:], ps, cos_sb[:, sl])
                nc.vector.tensor_mul(t2[0:64, :], ps[64:128, :], sin_sb[0:64, sl])
                nc.vector.tensor_mul(t2[64:128, :], ps[0:64, :], sin_sb[64:128, sl])
                nc.vector.tensor_add(dest, t1[:], t2[:])

            # ---- kv pass (own PSUM scope: 4 accs + transpose ring) ----
            with tc.tile_pool(name="s1psum", bufs=1, space="PSUM") as s1p:
                accs = [s1p.tile([128, 512], F32, tag=f"acc{i}", bufs=1,
                                 name=f"acckv{i}") for i in range(4)]
                for c in range(32):
                    wt = w0tiles[c // 2]
                    wof = 256 * (c % 2)
                    for i in range(4):
                        th = i % 2
                        wsl = slice(wof, wof + 128) if i < 2 \
                            else slice(wof + 128, wof + 256)
                        nc.tensor.matmul(accs[i][:], wt[:, wsl],
                                         hid(c, th),
                                         start=(c == 0), stop=(c == 31))
                for th in range(2):
                    sl = slice(512 * th, 512 * (th + 1))
                    rope(accs[th][:], sl, kT[:, sl], f"k{th}")
                    nc.scalar.copy(vT[:, sl], accs[2 + th][:])
                for tt in range(8):
                    tp = s1p.tile([128, 128], FP16, tag="tr", bufs=2,
                                  name=f"tp{tt}")
                    nc.tensor.transpose(tp[:], vT[:, 128 * tt:128 * (tt + 1)],
                                        id_sb[:])
                    nc.vector.tensor_copy(vnat[:, 128 * tt:128 * (tt + 1)],
                                          tp[:])

            with tc.tile_pool(name="psA", bufs=1, space="PSUM") as psA, \
                 tc.tile_pool(name="psB", bufs=1, space="PSUM") as psB:

                prev = [None]

                def q_pass(th, wtiles=None):
                    for p in range(1, 3):
                        acct = psA.tile([128, 1024], F32, tag="A", bufs=2,
                                        name=f"qacc{p}_{th}")
                        for c in range(16):
                            if wtiles is not None:
                                wt = wtiles[16 * (p - 1) + c]
                            else:
                                wt = ws.tile([128, 512], FP16, tag="w",
                                             name=f"w{p}_{th}_{c}")
                                nc.sync.dma_start(wt[:], wcat[p, c])
                            for half in range(2):
                                for fi in range(2):
                                    nc.tensor.matmul(
                                        acct[:, 512 * fi:512 * (fi + 1)],
                                        wt[:, 256 * half + 128 * fi:
                                           256 * half + 128 * (fi + 1)],
                                        hid(2 * c + half, th),
                                        start=(c == 0 and half == 0),
                                        stop=(c == 15 and half == 1))
                        for fi in range(2):
                            f = 2 * (p - 1) + fi
                            sl = slice(512 * th, 512 * (th + 1))
                            rope(acct[:, 512 * fi:512 * (fi + 1)], sl,
                                 qT[:, 1024 * f + 512 * th:
                                    1024 * f + 512 * (th + 1)], f"q{f}{th}")

                def flush_prev():
                    if prev[0] is None:
                        return
                    i, b, jt, expS, sumexp = prev[0]
                    prev[0] = None
                    # denominator: reduce sumexp over its 128 key-partitions
                    # AND broadcast to 128 partitions with one all-ones MM
                    rb_ps = psB.tile([128, 512], F32, tag="rbp", bufs=1,
                                     name=f"rbp{i}")
                    nc.tensor.matmul(rb_ps[:], ones_sb[:], sumexp[:])
                    rb_sb = rbp.tile([128, 512], F32, tag="rb", bufs=2,
                                     name=f"rb{i}")
                    nc.vector.reciprocal_approx_fast(rb_sb[:], rb_ps[:])
                    oTp = psB.tile([128, 512], F32, tag="oTp", bufs=2,
                                   name=f"oTp{i}")
                    for r in range(NKT):
                        ka = jt + r
                        if ka < 8:
                            lhs_v = vc_sb[:, CKEYS * b + 128 * ka:
                                          CKEYS * b + 128 * (ka + 1)]
                        else:
                            lhs_v = vnat[:, 128 * (2 * b + ka - 8):
                                         128 * (2 * b + ka - 7)]
                        nc.tensor.matmul(oTp[:], lhs_v,
                                         expS[:, 512 * r:512 * (r + 1)],
                                         start=(r == 0), stop=(r == NKT - 1))
                    oTp4 = oTp[:].rearrange("p (h t) -> p h t", h=HPC)
                    rb4 = rb_sb[:].rearrange("p (h t) -> p h t", h=HPC)
                    tsl = slice(256 * b + 128 * jt, 256 * b + 128 * (jt + 1))
                    nc.vector.tensor_mul(oT4[:, 0:HPC, tsl], oTp4[:, 0:HPC, :],
                                         rb4[:, 0:HPC, :])
                    blk = 2 * b + jt
                    ag_dst = ag_in[blk].rearrange("(h d) t -> d h t", h=HPC)
                    nc.gpsimd.dma_start(ag_dst[:, :, :], oT4[:, 0:HPC, tsl])
                    nc.gpsimd.collective_compute(
                        "AllGather", mybir.AluOpType.bypass,
                        replica_groups=[list(range(M))],
                        ins=[ag_in[blk].opt()], outs=[ag_out[blk].opt()])

                def emit_iter(i, b, jt):
                    rhs_q = qT4[:, 0:HPC,
                                256 * b + 128 * jt:256 * b + 128 * (jt + 1)]
                    expS = es.tile([128, NKT * 512], FP16, tag="expS",
                                   name=f"expS{i}")
                    r = 0
                    while r < NKT:
                        w = 2 if r + 1 < NKT else 1
                        sps = psA.tile([128, 1024], F32, tag="A", bufs=2,
                                       name=f"sps{i}_{r}")
                        for j in range(w):
                            ka = jt + r + j
                            if ka < 8:
                                lhs_k = kc_sb[:, CKEYS * b + 128 * ka:
                                              CKEYS * b + 128 * (ka + 1)]
                            else:
                                lhs_k = kT[:, 256 * b + 128 * (ka - 8):
                                           256 * b + 128 * (ka - 7)]
                            nc.tensor.matmul(sps[:, 512 * j:512 * (j + 1)],
                                             lhs_k, rhs_q)
                        nc.scalar.activation(expS[:, 512 * r:512 * (r + w)],
                                             sps[:, 0:512 * w],
                                             mybir.ActivationFunctionType.Exp)
                        if r == 0:
                            nc.vector.tensor_mul(expS[:, 0:512],
                                                 expS[:, 0:512],
                                                 mask_sb[:, 0:512])
                        if r + w == NKT:
                            nc.vector.tensor_mul(
                                expS[:, 512 * (NKT - 1):512 * NKT],
                                expS[:, 512 * (NKT - 1):512 * NKT],
                                mask_sb[:, 512:1024])
                        r += w
                    flush_prev()
                    # DVE tree-sum of the 9 masked expS tiles -> [128, 512]
                    a = trp.tile([128, 2048], FP16, tag="ta", bufs=2,
                                 name=f"ta{i}")
                    sumexp = trp.tile([128, 512], FP16, tag="ts", bufs=2,
                                      name=f"ts{i}")
                    nc.vector.tensor_add(a[:], expS[:, 0:2048],
                                         expS[:, 2048:4096])
                    nc.vector.tensor_add(a[:, 0:1024], a[:, 0:1024],
                                         a[:, 1024:2048])
                    nc.vector.tensor_add(a[:, 0:512], a[:, 0:512],
                                         a[:, 512:1024])
                    nc.vector.tensor_add(sumexp[:], a[:, 0:512],
                                         expS[:, 4096:4608])
                    prev[0] = (i, b, jt, expS, sumexp)

                def o_proj(blk):
                    # ocb shares the expS pool; gather is one rearranged DMA
                    # on the sync queue (idle by now; guard waits AG blk)
                    ocb = es.tile([128, NKT * 512], FP16, tag="expS",
                                  name=f"ocb{blk}")
                    nc.sync.dma_start(
                        ocb[:, 0:4096].rearrange("p (c t) -> p c t", c=32),
                        ag_out[blk].rearrange("(c p) t -> p c t", c=32))
                    out_ps = psB.tile([128, 512], F32, tag="oTp", bufs=2,
                                      name=f"outps{blk}")
                    for c in range(32):
                        nc.tensor.matmul(out_ps[:],
                                         ocb[:, 128 * c:128 * (c + 1)],
                                         wo_sb[:, 512 * c:512 * (c + 1)],
                                         start=(c == 0), stop=(c == 31))
                    osb = s3o.tile([128, 512], F32, tag="os", name=f"osb{blk}")
                    nc.vector.tensor_copy(osb[:], out_ps[:])
                    nc.sync.dma_start(outp[128 * blk:128 * (blk + 1), :],
                                      osb[:])

                q_pass(0)
                wq1tiles = []
                for it, (bb, jt) in enumerate([(0, 0), (0, 1), (1, 0), (1, 1)]):
                    emit_iter(it, bb, jt)
                    if it == 0:
                        # eager flush for iter 0: starts the AG pipe ~12us
                        # earlier at the cost of one small PE bubble
                        flush_prev()
                    # wo chunks + th1 q-weights stream on the idle sync queue
                    # during attention th0
                    for s in (2 * it, 2 * it + 1):
                        nc.sync.dma_start(
                            wo_sb[:, 2048 * s:2048 * (s + 1)]
                            .rearrange("p (c n) -> p c n", c=4),
                            wo[512 * s:512 * (s + 1), :]
                            .rearrange("(c p) n -> p c n", c=4))
                    for k in range(8 * it, 8 * (it + 1)):
                        p, c = 1 + k // 16, k % 16
                        wt = ws.tile([128, 512], FP16, tag="w",
                                     name=f"w{p}_1_{c}")
                        nc.sync.dma_start(wt[:], wcat[p, c])
                        wq1tiles.append(wt)
                flush_prev()
                q_pass(1, wtiles=wq1tiles)
                emit_iter(4, 2, 0)
                emit_iter(5, 2, 1)
                emit_iter(6, 3, 0)
                emit_iter(7, 3, 1)
                flush_prev()
                for blk in range(8):
                    o_proj(blk)

    nc.compile()
    return nc


def _prep_inputs(hidden_states, wq, wk, wv, wo, k_cache, v_cache,
                 position_ids, q_start_loc, q_seq_length, kv_seq_length,
                 block_offsets):
    f32 = np.float32
    fp16 = np.float16
    hidden_states = np.asarray(hidden_states, f32)
    position_ids = np.asarray(position_ids, np.int32)
    block_offsets = np.asarray(block_offsets, np.int32)

    hidT = np.ascontiguousarray(hidden_states.T).astype(fp16)  # [HID, T]

    # rope factors per (d, token)
    half = D // 2
    inv = 1.0 / (THETA ** (np.arange(half, dtype=f32) / half))
    f = position_ids.astype(f32)[:, None] * inv[None, :]            # [T, 64]
    cos = np.cos(f); sin = np.sin(f)
    cosT = np.ascontiguousarray(np.concatenate([cos, cos], 1).T)    # [128, T]
    sinTs = np.ascontiguousarray(np.concatenate([-sin, sin], 1).T)  # [128, T]

    # two static boundary masks [128 keys, 128 q], replicated x4 heads:
    # window lower edge (k > q strict) and causal edge (k <= q)
    ki = np.arange(128)[:, None]
    qi = np.arange(128)[None, :]
    mlow = (ki > qi).astype(fp16)
    mcau = (ki <= qi).astype(fp16)
    m2 = np.stack([np.tile(mlow, (1, 4)), np.tile(mcau, (1, 4))])  # [2,128,512]

    ident = np.eye(128, dtype=fp16)
    ones2d = np.ones((128, 128), fp16)

    blk0 = K0 // BS
    in_maps = []
    for m in range(M):
        # fold softmax scale into wq so exp needs no scale parameter
        wq_m = np.asarray(wq[:, 512 * m:512 * (m + 1)], f32) * SCALE
        wk_m = np.asarray(wk[:, 128 * m:128 * (m + 1)], f32)
        wv_m = np.asarray(wv[:, 128 * m:128 * (m + 1)], f32)
        # feature blocks in pass order: (k,v), (q0,q1), (q2,q3)
        fblocks = [wk_m, wv_m, wq_m[:, 0:128], wq_m[:, 128:256],
                   wq_m[:, 256:384], wq_m[:, 384:512]]
        wcat = np.empty((3, 16, 128, 512), fp16)
        for p in range(3):
            a = fblocks[2 * p].reshape(32, 128, 128)
            bb = fblocks[2 * p + 1].reshape(32, 128, 128)
            pair = np.concatenate([a, bb], axis=2)      # [32, 128, 256]
            wcat[p] = pair.reshape(16, 2, 128, 256) \
                .transpose(0, 2, 1, 3).reshape(16, 128, 512)
        wo_m = np.asarray(wo[:, 512 * m:512 * (m + 1)], f32).astype(fp16)

        kcT_m = np.empty((B, 128, CKEYS), fp16)
        vc_m = np.empty((B, 128, CKEYS), fp16)
        for b in range(B):
            blks = block_offsets[b, blk0:blk0 + NCBLK]
            kc = np.asarray(k_cache[blks, :, m, :], f32)     # [16, 64, 128]
            vcb = np.asarray(v_cache[blks, :, m, :], f32)
            kcT_m[b] = kc.reshape(CKEYS, 128).T              # [128 d, keys]
            vc_m[b] = vcb.reshape(8, 128, 128).transpose(1, 0, 2).reshape(128, CKEYS)
        in_maps.append(dict(
            hidT=hidT, wcat=wcat, wo=wo_m,
            kcT=np.ascontiguousarray(kcT_m), vc=np.ascontiguousarray(vc_m),
            cosT=cosT, sinTs=sinTs, masks=m2, ident=ident, ones2d=ones2d))
    return in_maps


def kernel(**inputs):
    in_maps = _prep_inputs(**inputs)
    if "nc" not in _CACHE:
        _CACHE["nc"] = _build()
    nc = _CACHE["nc"]

    kwargs = {}
    if os.environ.get("KERNEL_TRACE"):
        import types as _types
        from trn_agent_boot.trn_boot import _ntff_profile_via_ctypes
        hook = _ntff_profile_via_ctypes('/opt/axon/libaxon_pjrt.so')
        mod = _types.ModuleType("antenv.axon_hooks")
        mod.get_axon_ntff_profile_hook = lambda: hook
        sys.modules["antenv.axon_hooks"] = mod
        tdir = os.environ.get("KERNEL_TRACE_DIR", "/tmp/kernel_trace")
        os.makedirs(tdir, exist_ok=True)
        kwargs = dict(trace=True, tmpdir=tdir)

    res = run_bass_kernel_spmd(nc, in_maps, core_ids=list(range(M)), **kwargs)
    if res.exec_time_ns is not None:
        print(f"HW exec time: {res.exec_time_ns} ns")
    out = np.concatenate([res.results[m]["out"] for m in range(M)], axis=1)
    return np.ascontiguousarray(out, np.float32)


# revision 17
# speedup vs baseline: 1.0489x; 1.0489x over previous
"""Mistral flash-attention (paged KV, GQA, sliding window) on 8 TRN2 cores.

Tensor-parallel over heads: core m owns kv-head m and q-heads 4m..4m+3,
wq/wk/wv column-sharded, wo column-sharded; the attention output is
AllGathered in fp16 per 128-token block (pipelined with attention), then
each core computes its 512 output columns of o @ wo.

v4 layout: everything on-chip is fp16.  1/sqrt(D) is folded into wq on
the host.  Attention iterates per (batch, 128-query tile): the 1024-wide
sliding window needs 9 key tiles, and the boundary masks collapse to two
static 128x128 triangles.  The softmax denominator is computed by a DVE
tree-sum of the 9 expS tiles followed by a single all-ones matmul that
both reduces over keys and broadcasts the result to 128 partitions; the
reciprocal uses the fast fp32 custom-DVE op.  o_proj is interleaved with
the tail of attention: each 128-token block's 32 matmuls are emitted as
soon as its AllGather output can have landed, so the PE never drains at
the end.  The ocb gather tiles share the expS pool (SBUF stays under
budget with hid resident) and o_proj PSUM shares the oTp tag.  All
AG-gated DMA triggers live on the gpsimd queue so a stalled guard can
never block the exp/weight streams.  Input DMAs are ordered so the
hidden-state tiles stream first and the kv pass starts ~9us in.
"""
import os
import sys
import math
import numpy as np

import concourse.bacc as bacc
import concourse.tile as tile
from concourse import mybir
from concourse.bass_utils import run_bass_kernel_spmd

# ---- problem constants (hardcoded per contest rules) ----
HID = 4096; H = 32; KVH = 8; D = 128
B = 4; Q = 256; KV = 2048; HIST = KV - Q
BS = 64; NB = KV // BS; NBLOCKS = 160
WINDOW = 1024; THETA = 10000.0
T = B * Q                      # 1024 tokens
M = 8                          # cores
HPC = H // M                   # 4 q-heads per core
SCALE = 1.0 / math.sqrt(D)

# windowed cache key range: slots (HIST-WINDOW, HIST) come from the cache,
# slots [HIST, KV) are the new tokens computed on-chip.
K0 = HIST - WINDOW             # 768, first cache slot
NCBLK = (HIST - K0) // BS      # 16 cache blocks per seq
CKEYS = NCBLK * BS             # 1024 cache keys per seq
NKT = 9                        # key tiles of 128 per (batch, q-tile)

F32 = mybir.dt.float32
FP16 = mybir.dt.float16

_CACHE = {}


def _build():
    from contextlib import ExitStack
    nc = bacc.Bacc("TRN2", target_bir_lowering=False, debug=False,
                   enable_asserts=False, num_devices=M)

    dt_in = nc.dram_tensor
    hidT = dt_in("hidT", [HID, T], FP16, kind="ExternalInput").ap()
    # wcat[p, c] = [128, 512]: chunk-pair c of the two feature blocks of
    # pass p (kv / q01 / q23)
    wcat = dt_in("wcat", [3, 16, 128, 512], FP16, kind="ExternalInput").ap()
    wo = dt_in("wo", [HID, 512], FP16, kind="ExternalInput").ap()
    kcT = dt_in("kcT", [B, 128, CKEYS], FP16, kind="ExternalInput").ap()
    vc = dt_in("vc", [B, 128, CKEYS], FP16, kind="ExternalInput").ap()
    cosT = dt_in("cosT", [128, T], F32, kind="ExternalInput").ap()
    sinTs = dt_in("sinTs", [128, T], F32, kind="ExternalInput").ap()
    masks = dt_in("masks", [2, 128, 512], FP16, kind="ExternalInput").ap()
    ident = dt_in("ident", [128, 128], FP16, kind="ExternalInput").ap()
    ones2d = dt_in("ones2d", [128, 128], FP16, kind="ExternalInput").ap()
    outp = dt_in("out", [T, 512], F32, kind="ExternalOutput").ap()

    ag_in = [nc.dram_tensor(f"ag_in{s}", [512, 128], FP16).ap() for s in range(8)]
    ag_out = [nc.dram_tensor(f"ag_out{s}", [H * D, 128], FP16,
                             addr_space="Shared").ap() for s in range(8)]

    with tile.TileContext(nc) as tc, ExitStack() as top:
        persist = top.enter_context(tc.tile_pool(name="persist", bufs=1))

        qT = persist.tile([128, HPC * T], FP16, tag="qT")     # (head, token)
        kT = persist.tile([128, T], FP16, tag="kT")
        vnat = persist.tile([128, 8 * 128], FP16, tag="vnat")  # 8 token-tiles
        oT = persist.tile([128, HPC * T], FP16, tag="oT")
        ones_sb = persist.tile([128, 128], FP16, tag="ones2d")
        id_sb = persist.tile([128, 128], FP16, tag="ident")
        mask_sb = persist.tile([128, 2 * 512], FP16, tag="mask")
        wo_sb = persist.tile([128, 32 * 512], FP16, tag="wo_sb")
        kc_sb = persist.tile([128, B * CKEYS], FP16, tag="kc")
        vc_sb = persist.tile([128, B * CKEYS], FP16, tag="vc")

        # tiny warm-up AllGather so the first real AG doesn't pay
        # first-collective overhead; dram->dram copy so nothing gates on SBUF
        warm_in = nc.dram_tensor("warm_in", [1, 128], FP16).ap()
        warm_out = nc.dram_tensor("warm_out", [M, 128], FP16,
                                  addr_space="Shared").ap()
        nc.gpsimd.dma_start(warm_in[:, :], ident[0:1, 0:128])
        nc.gpsimd.collective_compute(
            "AllGather", mybir.AluOpType.bypass,
            replica_groups=[list(range(M))],
            ins=[warm_in.opt()], outs=[warm_out.opt()])

        qT4 = qT[:].rearrange("p (h t) -> p h t", h=HPC)
        oT4 = oT[:].rearrange("p (h t) -> p h t", h=HPC)

        with tc.tile_pool(name="s1", bufs=1) as s1, \
             tc.tile_pool(name="wstream", bufs=10) as ws, \
             tc.tile_pool(name="ropetmp", bufs=2) as rt, \
             tc.tile_pool(name="es", bufs=3) as es, \
             tc.tile_pool(name="tree", bufs=2) as trp, \
             tc.tile_pool(name="rbpool", bufs=2) as rbp, \
             tc.tile_pool(name="s3o", bufs=2) as s3o:

            # hidden-state tiles first on both DGE queues so the kv pass can
            # start as soon as chunk 0 lands
            # hid tiles + kv weights interleaved so chunk c's operands land
            # in consumption order: evens on scalar, odds on sync alternating
            # with the w0 chunk-pairs
            hidb = s1.tile([128, 32 * T], FP16, tag="hidb")

            def hid(c, th):
                return hidb[:, T * c + 512 * th:T * c + 512 * (th + 1)]
            w0tiles = []
            for c in range(32):
                dst = hidb[:, T * c:T * (c + 1)]
                src = hidT[128 * c:128 * (c + 1), :]
                if c % 2 == 0:
                    nc.scalar.dma_start(dst, src)
                    wt = ws.tile([128, 512], FP16, tag="w", name=f"w0_{c//2}")
                    nc.sync.dma_start(wt[:], wcat[0, c // 2])
                    w0tiles.append(wt)
                else:
                    nc.sync.dma_start(dst, src)
            # small constants on the gpsimd queue (idle early)
            nc.gpsimd.dma_start(id_sb[:], ident[:, :])
            nc.gpsimd.dma_start(ones_sb[:], ones2d[:, :])
            for i in range(2):
                nc.gpsimd.dma_start(mask_sb[:, 512 * i:512 * (i + 1)], masks[i])
            # rope factors + kv cache behind the hid stream on scalar
            cos_sb = s1.tile([128, T], F32, tag="cos")
            sin_sb = s1.tile([128, T], F32, tag="sin")
            nc.scalar.dma_start(cos_sb[:], cosT[:, :])
            nc.scalar.dma_start(sin_sb[:], sinTs[:, :])
            vT = s1.tile([128, T], FP16, tag="vT")
            for b in range(B):
                nc.scalar.dma_start(kc_sb[:, CKEYS * b:CKEYS * (b + 1)], kcT[b])
                nc.scalar.dma_start(vc_sb[:, CKEYS * b:CKEYS * (b + 1)], vc[b])

            def rope(ps, sl, dest, tag):
                t1 = rt.tile([128, 512], F32, tag="t1", name=f"t1{tag}")
                t2 = rt.tile([128, 512], F32, tag="t2", name=f"t2{tag}")
                nc.vector.tensor_mul(t1[:], ps, cos_sb[:, sl])
                nc.vector.tensor_mul(t2[0:64, :], ps[64:128, :], sin_sb[0:64, sl])
                nc.vector.tensor_mul(t2[64:128, :], ps[0:64, :], sin_sb[64:128, sl])
                nc.vector.tensor_add(dest, t1[:], t2[:])

            # ---- kv pass (own PSUM scope: 4 accs + transpose ring) ----
            with tc.tile_pool(name="s1psum", bufs=1, space="PSUM") as s1p:
                accs = [s1p.tile([128, 512], F32, tag=f"acc{i}", bufs=1,
                                 name=f"acckv{i}") for i in range(4)]
                for c in range(32):
                    wt = w0tiles[c // 2]
                    wof = 256 * (c % 2)
                    for i in range(4):
                        th = i % 2
                        wsl = slice(wof, wof + 128) if i < 2 \
                            else slice(wof + 128, wof + 256)
                        nc.tensor.matmul(accs[i][:], wt[:, wsl],
                                         hid(c, th),
                                         start=(c == 0), stop=(c == 31))
                for th in range(2):
                    sl = slice(512 * th, 512 * (th + 1))
                    rope(accs[th][:], sl, kT[:, sl], f"k{th}")
                    nc.scalar.copy(vT[:, sl], accs[2 + th][:])
                for tt in range(8):
                    tp = s1p.tile([128, 128], FP16, tag="tr", bufs=2,
                                  name=f"tp{tt}")
                    nc.tensor.transpose(tp[:], vT[:, 128 * tt:128 * (tt + 1)],
                                        id_sb[:])
                    nc.vector.tensor_copy(vnat[:, 128 * tt:128 * (tt + 1)],
                                          tp[:])

            with tc.tile_pool(name="psA", bufs=1, space="PSUM") as psA, \
                 tc.tile_pool(name="psB", bufs=1, space="PSUM") as psB:

                prev = [None]

                def q_pass(th, wtiles=None):
                    for p in range(1, 3):
                        acct = psA.tile([128, 1024], F32, tag="A", bufs=2,
                                        name=f"qacc{p}_{th}")
                        for c in range(16):
                            if wtiles is not None:
                                wt = wtiles[16 * (p - 1) + c]
                            else:
                                wt = ws.tile([128, 512], FP16, tag="w",
                                             name=f"w{p}_{th}_{c}")
                                nc.sync.dma_start(wt[:], wcat[p, c])
                            for half in range(2):
                                for fi in range(2):
                                    nc.tensor.matmul(
                                        acct[:, 512 * fi:512 * (fi + 1)],
                                        wt[:, 256 * half + 128 * fi:
                                           256 * half + 128 * (fi + 1)],
                                        hid(2 * c + half, th),
                                        start=(c == 0 and half == 0),
                                        stop=(c == 15 and half == 1))
                        for fi in range(2):
                            f = 2 * (p - 1) + fi
                            sl = slice(512 * th, 512 * (th + 1))
                            rope(acct[:, 512 * fi:512 * (fi + 1)], sl,
                                 qT[:, 1024 * f + 512 * th:
                                    1024 * f + 512 * (th + 1)], f"q{f}{th}")

                def flush_prev():
                    if prev[0] is None:
                        return
                    i, b, jt, expS, sumexp = prev[0]
                    prev[0] = None
                    # denominator: reduce sumexp over its 128 key-partitions
                    # AND broadcast to 128 partitions with one all-ones MM
                    rb_ps = psB.tile([128, 512], F32, tag="rbp", bufs=1,
                                     name=f"rbp{i}")
                    nc.tensor.matmul(rb_ps[:], ones_sb[:], sumexp[:])
                    rb_sb = rbp.tile([128, 512], F32, tag="rb", bufs=2,
                                     name=f"rb{i}")
                    nc.vector.reciprocal_approx_fast(rb_sb[:], rb_ps[:])
                    oTp = psB.tile([128, 512], F32, tag="oTp", bufs=2,
                                   name=f"oTp{i}")
                    for r in range(NKT):
                        ka = jt + r
                        if ka < 8:
                            lhs_v = vc_sb[:, CKEYS * b + 128 * ka:
                                          CKEYS * b + 128 * (ka + 1)]
                        else:
                            lhs_v = vnat[:, 128 * (2 * b + ka - 8):
                                         128 * (2 * b + ka - 7)]
                        nc.tensor.matmul(oTp[:], lhs_v,
                                         expS[:, 512 * r:512 * (r + 1)],
                                         start=(r == 0), stop=(r == NKT - 1))
                    oTp4 = oTp[:].rearrange("p (h t) -> p h t", h=HPC)
                    rb4 = rb_sb[:].rearrange("p (h t) -> p h t", h=HPC)
                    tsl = slice(256 * b + 128 * jt, 256 * b + 128 * (jt + 1))
                    nc.vector.tensor_mul(oT4[:, 0:HPC, tsl], oTp4[:, 0:HPC, :],
                                         rb4[:, 0:HPC, :])
                    blk = 2 * b + jt
                    ag_dst = ag_in[blk].rearrange("(h d) t -> d h t", h=HPC)
                    nc.gpsimd.dma_start(ag_dst[:, :, :], oT4[:, 0:HPC, tsl])
                    nc.gpsimd.collective_compute(
                        "AllGather", mybir.AluOpType.bypass,
                        replica_groups=[list(range(M))],
                        ins=[ag_in[blk].opt()], outs=[ag_out[blk].opt()])

                def emit_iter(i, b, jt):
                    rhs_q = qT4[:, 0:HPC,
                                256 * b + 128 * jt:256 * b + 128 * (jt + 1)]
                    expS = es.tile([128, NKT * 512], FP16, tag="expS",
                                   name=f"expS{i}")
                    r = 0
                    while r < NKT:
                        w = 2 if r + 1 < NKT else 1
                        sps = psA.tile([128, 1024], F32, tag="A", bufs=2,
                                       name=f"sps{i}_{r}")
                        for j in range(w):
                            ka = jt + r + j
                            if ka < 8:
                                lhs_k = kc_sb[:, CKEYS * b + 128 * ka:
                                              CKEYS * b + 128 * (ka + 1)]
                            else:
                                lhs_k = kT[:, 256 * b + 128 * (ka - 8):
                                           256 * b + 128 * (ka - 7)]
                            nc.tensor.matmul(sps[:, 512 * j:512 * (j + 1)],
                                             lhs_k, rhs_q)
                        nc.scalar.activation(expS[:, 512 * r:512 * (r + w)],
                                             sps[:, 0:512 * w],
                                             mybir.ActivationFunctionType.Exp)
                        if r == 0:
                            nc.vector.tensor_mul(expS[:, 0:512],
                                                 expS[:, 0:512],
                                                 mask_sb[:, 0:512])
                        if r + w == NKT:
                            nc.vector.tensor_mul(
                                expS[:, 512 * (NKT - 1):512 * NKT],
                                expS[:, 512 * (NKT - 1):512 * NKT],
                                mask_sb[:, 512:1024])
                        r += w
                    flush_prev()
                    # DVE tree-sum of the 9 masked expS tiles -> [128, 512]
                    a = trp.tile([128, 2048], FP16, tag="ta", bufs=2,
                                 name=f"ta{i}")
                    sumexp = trp.tile([128, 512], FP16, tag="ts", bufs=2,
                                      name=f"ts{i}")
                    nc.vector.tensor_add(a[:], expS[:, 0:2048],
                                         expS[:, 2048:4096])
                    nc.vector.tensor_add(a[:, 0:1024], a[:, 0:1024],
                                         a[:, 1024:2048])
                    nc.vector.tensor_add(a[:, 0:512], a[:, 0:512],
                                         a[:, 512:1024])
                    nc.vector.tensor_add(sumexp[:], a[:, 0:512],
                                         expS[:, 4096:4608])
                    prev[0] = (i, b, jt, expS, sumexp)

                def o_proj(blk):
                    # ocb shares the expS pool; gather is one rearranged DMA
                    # on the sync queue (idle by now; guard waits AG blk)
                    ocb = es.tile([128, NKT * 512], FP16, tag="expS",
                                  name=f"ocb{blk}")
                    nc.sync.dma_start(
                        ocb[:, 0:4096].rearrange("p (c t) -> p c t", c=32),
                        ag_out[blk].rearrange("(c p) t -> p c t", c=32))
                    out_ps = psB.tile([128, 512], F32, tag="oTp", bufs=2,
                                      name=f"outps{blk}")
                    for c in range(32):
                        nc.tensor.matmul(out_ps[:],
                                         ocb[:, 128 * c:128 * (c + 1)],
                                         wo_sb[:, 512 * c:512 * (c + 1)],
                                         start=(c == 0), stop=(c == 31))
                    osb = s3o.tile([128, 512], F32, tag="os", name=f"osb{blk}")
                    nc.vector.tensor_copy(osb[:], out_ps[:])
                    nc.sync.dma_start(outp[128 * blk:128 * (blk + 1), :],
                                      osb[:])

                q_pass(0)
                wq1tiles = []
                for it, (bb, jt) in enumerate([(0, 0), (0, 1), (1, 0), (1, 1)]):
                    emit_iter(it, bb, jt)
                    if it == 0:
                        # eager flush for iter 0: starts the AG pipe ~12us
                        # earlier at the cost of one small PE bubble
                        flush_prev()
                    # wo chunks + th1 q-weights stream on the idle sync queue
                    # during attention th0
                    for s in (2 * it, 2 * it + 1):
                        nc.sync.dma_start(
                            wo_sb[:, 2048 * s:2048 * (s + 1)]
                            .rearrange("p (c n) -> p c n", c=4),
                            wo[512 * s:512 * (s + 1), :]
                            .rearrange("(c p) n -> p c n", c=4))
                    for k in range(8 * it, 8 * (it + 1)):
                        p, c = 1 + k // 16, k % 16
                        wt = ws.tile([128, 512], FP16, tag="w",
                                     name=f"w{p}_1_{c}")
                        nc.sync.dma_start(wt[:], wcat[p, c])
                        wq1tiles.append(wt)
                flush_prev()
                q_pass(1, wtiles=wq1tiles)
                emit_iter(4, 2, 0)
                emit_iter(5, 2, 1)
                emit_iter(6, 3, 0)
                emit_iter(7, 3, 1)
                flush_prev()
                for blk in range(8):
                    o_proj(blk)

    nc.compile()
    return nc


def _prep_inputs(hidden_states, wq, wk, wv, wo, k_cache, v_cache,
                 position_ids, q_start_loc, q_seq_length, kv_seq_length,
                 block_offsets):
    f32 = np.float32
    fp16 = np.float16
    hidden_states = np.asarray(hidden_states, f32)
    position_ids = np.asarray(position_ids, np.int32)
    block_offsets = np.asarray(block_offsets, np.int32)

    hidT = np.ascontiguousarray(hidden_states.T).astype(fp16)  # [HID, T]

    # rope factors per (d, token)
    half = D // 2
    inv = 1.0 / (THETA ** (np.arange(half, dtype=f32) / half))
    f = position_ids.astype(f32)[:, None] * inv[None, :]            # [T, 64]
    cos = np.cos(f); sin = np.sin(f)
    cosT = np.ascontiguousarray(np.concatenate([cos, cos], 1).T)    # [128, T]
    sinTs = np.ascontiguousarray(np.concatenate([-sin, sin], 1).T)  # [128, T]

    # two static boundary masks [128 keys, 128 q], replicated x4 heads:
    # window lower edge (k > q strict) and causal edge (k <= q)
    ki = np.arange(128)[:, None]
    qi = np.arange(128)[None, :]
    mlow = (ki > qi).astype(fp16)
    mcau = (ki <= qi).astype(fp16)
    m2 = np.stack([np.tile(mlow, (1, 4)), np.tile(mcau, (1, 4))])  # [2,128,512]

    ident = np.eye(128, dtype=fp16)
    ones2d = np.ones((128, 128), fp16)

    blk0 = K0 // BS
    in_maps = []
    for m in range(M):
        # fold softmax scale into wq so exp needs no scale parameter
        wq_m = np.asarray(wq[:, 512 * m:512 * (m + 1)], f32) * SCALE
        wk_m = np.asarray(wk[:, 128 * m:128 * (m + 1)], f32)
        wv_m = np.asarray(wv[:, 128 * m:128 * (m + 1)], f32)
        # feature blocks in pass order: (k,v), (q0,q1), (q2,q3)
        fblocks = [wk_m, wv_m, wq_m[:, 0:128], wq_m[:, 128:256],
                   wq_m[:, 256:384], wq_m[:, 384:512]]
        wcat = np.empty((3, 16, 128, 512), fp16)
        for p in range(3):
            a = fblocks[2 * p].reshape(32, 128, 128)
            bb = fblocks[2 * p + 1].reshape(32, 128, 128)
            pair = np.concatenate([a, bb], axis=2)      # [32, 128, 256]
            wcat[p] = pair.reshape(16, 2, 128, 256) \
                .transpose(0, 2, 1, 3).reshape(16, 128, 512)
        wo_m = np.asarray(wo[:, 512 * m:512 * (m + 1)], f32).astype(fp16)

        kcT_m = np.empty((B, 128, CKEYS), fp16)
        vc_m = np.empty((B, 128, CKEYS), fp16)
        for b in range(B):
            blks = block_offsets[b, blk0:blk0 + NCBLK]
            kc = np.asarray(k_cache[blks, :, m, :], f32)     # [16, 64, 128]
            vcb = np.asarray(v_cache[blks, :, m, :], f32)
            kcT_m[b] = kc.reshape(CKEYS, 128).T              # [128 d, keys]
            vc_m[b] = vcb.reshape(8, 128, 128).transpose(1, 0, 2).reshape(128, CKEYS)
        in_maps.append(dict(
            hidT=hidT, wcat=wcat, wo=wo_m,
            kcT=np.ascontiguousarray(kcT_m), vc=np.ascontiguousarray(vc_m),
            cosT=cosT, sinTs=sinTs, masks=m2, ident=ident, ones2d=ones2d))
    return in_maps


def kernel(**inputs):
    in_maps = _prep_inputs(**inputs)
    if "nc" not in _CACHE:
        _CACHE["nc"] = _build()
    nc = _CACHE["nc"]

    kwargs = {}
    if os.environ.get("KERNEL_TRACE"):
        import types as _types
        from trn_agent_boot.trn_boot import _ntff_profile_via_ctypes
        hook = _ntff_profile_via_ctypes('/opt/axon/libaxon_pjrt.so')
        mod = _types.ModuleType("antenv.axon_hooks")
        mod.get_axon_ntff_profile_hook = lambda: hook
        sys.modules["antenv.axon_hooks"] = mod
        tdir = os.environ.get("KERNEL_TRACE_DIR", "/tmp/kernel_trace")
        os.makedirs(tdir, exist_ok=True)
        kwargs = dict(trace=True, tmpdir=tdir)

    res = run_bass_kernel_spmd(nc, in_maps, core_ids=list(range(M)), **kwargs)
    if res.exec_time_ns is not None:
        print(f"HW exec time: {res.exec_time_ns} ns")
    out = np.concatenate([res.results[m]["out"] for m in range(M)], axis=1)
    return np.ascontiguousarray(out, np.float32)


# revision 27
# speedup vs baseline: 1.1017x; 1.0504x over previous
"""Mistral flash-attention (paged KV, GQA, sliding window) on 8 TRN2 cores.

Tensor-parallel over heads: core m owns kv-head m and q-heads 4m..4m+3,
wq/wk/wv column-sharded, wo column-sharded; the attention output is
AllGathered in fp16 per 128-token block (pipelined with attention), then
each core computes its 512 output columns of o @ wo.

v4 layout: everything on-chip is fp16.  1/sqrt(D) is folded into wq on
the host.  Attention iterates per (batch, 128-query tile): the 1024-wide
sliding window needs 9 key tiles, and the boundary masks collapse to two
static 128x128 triangles.  The softmax denominator is computed by a DVE
tree-sum of the 9 expS tiles followed by a single all-ones matmul that
both reduces over keys and broadcasts the result to 128 partitions; the
reciprocal uses the fast fp32 custom-DVE op.  o_proj is interleaved with
the tail of attention: each 128-token block's 32 matmuls are emitted as
soon as its AllGather output can have landed, so the PE never drains at
the end.  The ocb gather tiles share the expS pool (SBUF stays under
budget with hid resident) and o_proj PSUM shares the oTp tag.  All
AG-gated DMA triggers live on the gpsimd queue so a stalled guard can
never block the exp/weight streams.  Input DMAs are ordered so the
hidden-state tiles stream first and the kv pass starts ~9us in.
"""
import os
import sys
import math
import numpy as np

import concourse.bacc as bacc
import concourse.tile as tile
from concourse import mybir
from concourse.bass_utils import run_bass_kernel_spmd

# ---- problem constants (hardcoded per contest rules) ----
HID = 4096; H = 32; KVH = 8; D = 128
B = 4; Q = 256; KV = 2048; HIST = KV - Q
BS = 64; NB = KV // BS; NBLOCKS = 160
WINDOW = 1024; THETA = 10000.0
T = B * Q                      # 1024 tokens
M = 8                          # cores
HPC = H // M                   # 4 q-heads per core
SCALE = 1.0 / math.sqrt(D)

# windowed cache key range: slots (HIST-WINDOW, HIST) come from the cache,
# slots [HIST, KV) are the new tokens computed on-chip.
K0 = HIST - WINDOW             # 768, first cache slot
NCBLK = (HIST - K0) // BS      # 16 cache blocks per seq
CKEYS = NCBLK * BS             # 1024 cache keys per seq
NKT = 9                        # key tiles of 128 per (batch, q-tile)

F32 = mybir.dt.float32
FP16 = mybir.dt.float16

_CACHE = {}


def _build():
    from contextlib import ExitStack
    nc = bacc.Bacc("TRN2", target_bir_lowering=False, debug=False,
                   enable_asserts=False, num_devices=M)

    dt_in = nc.dram_tensor
    hidT = dt_in("hidT", [HID, T], FP16, kind="ExternalInput").ap()
    # wcat[p, c] = [128, 512]: chunk-pair c of the two feature blocks of
    # pass p (kv / q01 / q23)
    wcat = dt_in("wcat", [3, 16, 128, 512], FP16, kind="ExternalInput").ap()
    wo = dt_in("wo", [HID, 512], FP16, kind="ExternalInput").ap()
    kcT = dt_in("kcT", [B, 128, CKEYS], FP16, kind="ExternalInput").ap()
    vc = dt_in("vc", [B, 128, CKEYS], FP16, kind="ExternalInput").ap()
    cosT = dt_in("cosT", [128, T], FP16, kind="ExternalInput").ap()
    sinTs = dt_in("sinTs", [128, T], FP16, kind="ExternalInput").ap()
    masks = dt_in("masks", [2, 128, 512], FP16, kind="ExternalInput").ap()
    ident = dt_in("ident", [128, 128], FP16, kind="ExternalInput").ap()
    ones2d = dt_in("ones2d", [128, 128], FP16, kind="ExternalInput").ap()
    outp = dt_in("out", [T, 512], F32, kind="ExternalOutput").ap()

    ag_in = [nc.dram_tensor(f"ag_in{s}", [512, 128], FP16).ap() for s in range(8)]
    ag_out = [nc.dram_tensor(f"ag_out{s}", [H * D, 128], FP16,
                             addr_space="Shared").ap() for s in range(8)]

    with tile.TileContext(nc) as tc, ExitStack() as top:
        persist = top.enter_context(tc.tile_pool(name="persist", bufs=1))

        qT = persist.tile([128, HPC * T], FP16, tag="qT")     # (head, token)
        kT = persist.tile([128, T], FP16, tag="kT")
        vnat = persist.tile([128, 8 * 128], FP16, tag="vnat")  # 8 token-tiles
        oT = persist.tile([128, HPC * T], FP16, tag="oT")
        ones_sb = persist.tile([128, 128], FP16, tag="ones2d")
        id_sb = persist.tile([128, 128], FP16, tag="ident")
        mask_sb = persist.tile([128, 2 * 512], FP16, tag="mask")
        wo_sb = persist.tile([128, 32 * 512], FP16, tag="wo_sb")
        kc_sb = persist.tile([128, B * CKEYS], FP16, tag="kc")
        vc_sb = persist.tile([128, B * CKEYS], FP16, tag="vc")

        # tiny warm-up AllGather so the first real AG doesn't pay
        # first-collective overhead; dram->dram copy so nothing gates on SBUF
        warm_in = nc.dram_tensor("warm_in", [1, 128], FP16).ap()
        warm_out = nc.dram_tensor("warm_out", [M, 128], FP16,
                                  addr_space="Shared").ap()
        nc.gpsimd.dma_start(warm_in[:, :], ident[0:1, 0:128])
        nc.gpsimd.collective_compute(
            "AllGather", mybir.AluOpType.bypass,
            replica_groups=[list(range(M))],
            ins=[warm_in.opt()], outs=[warm_out.opt()])

        qT4 = qT[:].rearrange("p (h t) -> p h t", h=HPC)
        oT4 = oT[:].rearrange("p (h t) -> p h t", h=HPC)

        with tc.tile_pool(name="s1", bufs=1) as s1, \
             tc.tile_pool(name="wstream", bufs=20) as ws, \
             tc.tile_pool(name="ropetmp", bufs=2) as rt, \
             tc.tile_pool(name="es", bufs=3) as es, \
             tc.tile_pool(name="tree", bufs=2) as trp, \
             tc.tile_pool(name="rbpool", bufs=2) as rbp, \
             tc.tile_pool(name="s3o", bufs=2) as s3o:

            # hidden-state tiles first on both DGE queues so the kv pass can
            # start as soon as chunk 0 lands
            # hid tiles + kv weights interleaved so chunk c's operands land
            # in consumption order: evens on scalar, odds on sync alternating
            # with the w0 chunk-pairs
            hidb = s1.tile([128, 32 * T], FP16, tag="hidb")

            def hid(c, th):
                return hidb[:, T * c + 512 * th:T * c + 512 * (th + 1)]
            w0tiles = []
            for c in range(32):
                dst = hidb[:, T * c:T * (c + 1)]
                src = hidT[128 * c:128 * (c + 1), :]
                if c % 2 == 0:
                    nc.scalar.dma_start(dst, src)
                    wt = ws.tile([128, 512], FP16, tag="w", name=f"w0_{c//2}")
                    nc.sync.dma_start(wt[:], wcat[0, c // 2])
                    w0tiles.append(wt)
                else:
                    nc.sync.dma_start(dst, src)
            # small constants on the gpsimd queue (idle early)
            nc.gpsimd.dma_start(id_sb[:], ident[:, :])
            nc.gpsimd.dma_start(ones_sb[:], ones2d[:, :])
            for i in range(2):
                nc.gpsimd.dma_start(mask_sb[:, 512 * i:512 * (i + 1)], masks[i])
            # rope factors + kv cache behind the hid stream on scalar
            cos_sb = s1.tile([128, T], FP16, tag="cos")
            sin_sb = s1.tile([128, T], FP16, tag="sin")
            nc.scalar.dma_start(cos_sb[:], cosT[:, :])
            nc.scalar.dma_start(sin_sb[:], sinTs[:, :])
            for b in range(B):
                nc.scalar.dma_start(kc_sb[:, CKEYS * b:CKEYS * (b + 1)], kcT[b])
                nc.scalar.dma_start(vc_sb[:, CKEYS * b:CKEYS * (b + 1)], vc[b])


            def rope(ps, sl, dest, tag):
                t1 = rt.tile([128, 512], F32, tag="t1", bufs=1, name=f"t1{tag}")
                t2 = rt.tile([128, 512], F32, tag="t2", bufs=1, name=f"t2{tag}")
                nc.vector.tensor_mul(t1[:], ps, cos_sb[:, sl])
                nc.vector.tensor_mul(t2[0:64, :], ps[64:128, :], sin_sb[0:64, sl])
                nc.vector.tensor_mul(t2[64:128, :], ps[0:64, :], sin_sb[64:128, sl])
                nc.vector.tensor_add(dest, t1[:], t2[:])

            # ---- kv pass (own PSUM scope: 4 accs + transpose ring) ----
            with tc.tile_pool(name="s1psum", bufs=1, space="PSUM") as s1p, \
                 tc.tile_pool(name="kvsb", bufs=1) as kvsb:
                vT = kvsb.tile([128, T], FP16, tag="vT")
                accs = [s1p.tile([128, 512], F32, tag=f"acc{i}", bufs=1,
                                 name=f"acckv{i}") for i in range(4)]
                for c in range(32):
                    wt = w0tiles[c // 2]
                    wof = 256 * (c % 2)
                    for i in range(4):
                        th = i % 2
                        wsl = slice(wof, wof + 128) if i < 2 \
                            else slice(wof + 128, wof + 256)
                        nc.tensor.matmul(accs[i][:], wt[:, wsl],
                                         hid(c, th),
                                         start=(c == 0), stop=(c == 31))
                for th in range(2):
                    sl = slice(512 * th, 512 * (th + 1))
                    rope(accs[th][:], sl, kT[:, sl], f"k{th}")
                    nc.scalar.copy(vT[:, sl], accs[2 + th][:])
                for tt in range(8):
                    tp = s1p.tile([128, 128], FP16, tag="tr", bufs=2,
                                  name=f"tp{tt}")
                    nc.tensor.transpose(tp[:], vT[:, 128 * tt:128 * (tt + 1)],
                                        id_sb[:])
                    nc.vector.tensor_copy(vnat[:, 128 * tt:128 * (tt + 1)],
                                          tp[:])

            with tc.tile_pool(name="psA", bufs=1, space="PSUM") as psA, \
                 tc.tile_pool(name="psB", bufs=1, space="PSUM") as psB:

                prev = [None]

                def q_pass(th, wtiles=None):
                    for p in range(1, 3):
                        acct = psA.tile([128, 1024], F32, tag="A", bufs=2,
                                        name=f"qacc{p}_{th}")
                        for c in range(16):
                            if wtiles is not None:
                                wt = wtiles[16 * (p - 1) + c]
                            else:
                                wt = ws.tile([128, 512], FP16, tag="w",
                                             name=f"w{p}_{th}_{c}")
                                nc.sync.dma_start(wt[:], wcat[p, c])
                            for half in range(2):
                                for fi in range(2):
                                    nc.tensor.matmul(
                                        acct[:, 512 * fi:512 * (fi + 1)],
                                        wt[:, 256 * half + 128 * fi:
                                           256 * half + 128 * (fi + 1)],
                                        hid(2 * c + half, th),
                                        start=(c == 0 and half == 0),
                                        stop=(c == 15 and half == 1))
                        for fi in range(2):
                            f = 2 * (p - 1) + fi
                            sl = slice(512 * th, 512 * (th + 1))
                            rope(acct[:, 512 * fi:512 * (fi + 1)], sl,
                                 qT[:, 1024 * f + 512 * th:
                                    1024 * f + 512 * (th + 1)], f"q{f}{th}")

                def flush_prev():
                    if prev[0] is None:
                        return
                    i, b, jt, expS, sumexp = prev[0]
                    prev[0] = None
                    # denominator: reduce sumexp over its 128 key-partitions
                    # AND broadcast to 128 partitions with one all-ones MM
                    rb_ps = psB.tile([128, 512], F32, tag="rbp", bufs=1,
                                     name=f"rbp{i}")
                    nc.tensor.matmul(rb_ps[:], ones_sb[:], sumexp[:])
                    rb_sb = rbp.tile([128, 512], F32, tag="rb", bufs=2,
                                     name=f"rb{i}")
                    nc.vector.reciprocal_approx_fast(rb_sb[:], rb_ps[:])
                    oTp = psB.tile([128, 512], F32, tag="oTp", bufs=2,
                                   name=f"oTp{i}")
                    for r in range(NKT):
                        ka = jt + r
                        if ka < 8:
                            lhs_v = vc_sb[:, CKEYS * b + 128 * ka:
                                          CKEYS * b + 128 * (ka + 1)]
                        else:
                            lhs_v = vnat[:, 128 * (2 * b + ka - 8):
                                         128 * (2 * b + ka - 7)]
                        nc.tensor.matmul(oTp[:], lhs_v,
                                         expS[:, 512 * r:512 * (r + 1)],
                                         start=(r == 0), stop=(r == NKT - 1))
                    oTp4 = oTp[:].rearrange("p (h t) -> p h t", h=HPC)
                    rb4 = rb_sb[:].rearrange("p (h t) -> p h t", h=HPC)
                    tsl = slice(256 * b + 128 * jt, 256 * b + 128 * (jt + 1))
                    nc.vector.tensor_mul(oT4[:, 0:HPC, tsl], oTp4[:, 0:HPC, :],
                                         rb4[:, 0:HPC, :])
                    blk = 2 * b + jt
                    ag_dst = ag_in[blk].rearrange("(h d) t -> d h t", h=HPC)
                    nc.gpsimd.dma_start(ag_dst[:, :, :], oT4[:, 0:HPC, tsl])
                    nc.gpsimd.collective_compute(
                        "AllGather", mybir.AluOpType.bypass,
                        replica_groups=[list(range(M))],
                        ins=[ag_in[blk].opt()], outs=[ag_out[blk].opt()])

                def emit_iter(i, b, jt):
                    rhs_q = qT4[:, 0:HPC,
                                256 * b + 128 * jt:256 * b + 128 * (jt + 1)]
                    expS = es.tile([128, NKT * 512], FP16, tag="expS",
                                   name=f"expS{i}")
                    r = 0
                    while r < NKT:
                        w = 2 if r + 1 < NKT else 1
                        sps = psA.tile([128, 1024], F32, tag="A", bufs=2,
                                       name=f"sps{i}_{r}")
                        for j in range(w):
                            ka = jt + r + j
                            if ka < 8:
                                lhs_k = kc_sb[:, CKEYS * b + 128 * ka:
                                              CKEYS * b + 128 * (ka + 1)]
                            else:
                                lhs_k = kT[:, 256 * b + 128 * (ka - 8):
                                           256 * b + 128 * (ka - 7)]
                            nc.tensor.matmul(sps[:, 512 * j:512 * (j + 1)],
                                             lhs_k, rhs_q)
                        nc.scalar.activation(expS[:, 512 * r:512 * (r + w)],
                                             sps[:, 0:512 * w],
                                             mybir.ActivationFunctionType.Exp)
                        if r == 0:
                            nc.vector.tensor_mul(expS[:, 0:512],
                                                 expS[:, 0:512],
                                                 mask_sb[:, 0:512])
                        if r + w == NKT:
                            nc.vector.tensor_mul(
                                expS[:, 512 * (NKT - 1):512 * NKT],
                                expS[:, 512 * (NKT - 1):512 * NKT],
                                mask_sb[:, 512:1024])
                        r += w
                    flush_prev()
                    # DVE tree-sum of the 9 masked expS tiles -> [128, 512]
                    a = trp.tile([128, 2048], FP16, tag="ta", bufs=1,
                                 name=f"ta{i}")
                    sumexp = trp.tile([128, 512], FP16, tag="ts", bufs=2,
                                      name=f"ts{i}")
                    nc.vector.tensor_add(a[:], expS[:, 0:2048],
                                         expS[:, 2048:4096])
                    nc.vector.tensor_add(a[:, 0:1024], a[:, 0:1024],
                                         a[:, 1024:2048])
                    nc.vector.tensor_add(a[:, 0:512], a[:, 0:512],
                                         a[:, 512:1024])
                    nc.vector.tensor_add(sumexp[:], a[:, 0:512],
                                         expS[:, 4096:4608])
                    prev[0] = (i, b, jt, expS, sumexp)

                def o_proj(blk):
                    # ocb shares the expS pool; gather is one rearranged DMA
                    # on the sync queue (idle by now; guard waits AG blk)
                    ocb = es.tile([128, NKT * 512], FP16, tag="expS",
                                  name=f"ocb{blk}")
                    nc.sync.dma_start(
                        ocb[:, 0:4096].rearrange("p (c t) -> p c t", c=32),
                        ag_out[blk].rearrange("(c p) t -> p c t", c=32))
                    out_ps = psB.tile([128, 512], F32, tag="oTp", bufs=2,
                                      name=f"outps{blk}")
                    for c in range(32):
                        nc.tensor.matmul(out_ps[:],
                                         ocb[:, 128 * c:128 * (c + 1)],
                                         wo_sb[:, 512 * c:512 * (c + 1)],
                                         start=(c == 0), stop=(c == 31))
                    osb = s3o.tile([128, 512], F32, tag="os", name=f"osb{blk}")
                    nc.vector.tensor_copy(osb[:], out_ps[:])
                    nc.sync.dma_start(outp[128 * blk:128 * (blk + 1), :],
                                      osb[:])

                q_pass(0)
                # th1 q-weights + wo stream on sync in the pre-pipe lull:
                # their ws-pool credits clear as q_pass(0) consumes tiles, and
                # nothing latency-critical sits behind them on the sync queue
                wq1tiles = []
                for k in range(32):
                    p, c = 1 + k // 16, k % 16
                    wt = ws.tile([128, 512], FP16, tag="w", name=f"w{p}_1_{c}")
                    nc.sync.dma_start(wt[:], wcat[p, c])
                    wq1tiles.append(wt)
                for s in range(8):
                    nc.sync.dma_start(
                        wo_sb[:, 2048 * s:2048 * (s + 1)]
                        .rearrange("p (c n) -> p c n", c=4),
                        wo[512 * s:512 * (s + 1), :]
                        .rearrange("(c p) n -> p c n", c=4))
                for it, (bb, jt) in enumerate([(0, 0), (0, 1), (1, 0), (1, 1)]):
                    emit_iter(it, bb, jt)
                    if it == 0:
                        # eager flush for iter 0: starts the AG pipe ~12us
                        # earlier at the cost of one small PE bubble
                        flush_prev()
                flush_prev()
                q_pass(1, wtiles=wq1tiles)
                emit_iter(4, 2, 0)
                emit_iter(5, 2, 1)
                emit_iter(6, 3, 0)
                emit_iter(7, 3, 1)
                flush_prev()
                for blk in range(8):
                    o_proj(blk)

    nc.compile()
    return nc


def _prep_inputs(hidden_states, wq, wk, wv, wo, k_cache, v_cache,
                 position_ids, q_start_loc, q_seq_length, kv_seq_length,
                 block_offsets):
    f32 = np.float32
    fp16 = np.float16
    hidden_states = np.asarray(hidden_states, f32)
    position_ids = np.asarray(position_ids, np.int32)
    block_offsets = np.asarray(block_offsets, np.int32)

    hidT = np.ascontiguousarray(hidden_states.T).astype(fp16)  # [HID, T]

    # rope factors per (d, token)
    half = D // 2
    inv = 1.0 / (THETA ** (np.arange(half, dtype=f32) / half))
    f = position_ids.astype(f32)[:, None] * inv[None, :]            # [T, 64]
    cos = np.cos(f); sin = np.sin(f)
    cosT = np.ascontiguousarray(np.concatenate([cos, cos], 1).T).astype(fp16)
    sinTs = np.ascontiguousarray(np.concatenate([-sin, sin], 1).T).astype(fp16)

    # two static boundary masks [128 keys, 128 q], replicated x4 heads:
    # window lower edge (k > q strict) and causal edge (k <= q)
    ki = np.arange(128)[:, None]
    qi = np.arange(128)[None, :]
    mlow = (ki > qi).astype(fp16)
    mcau = (ki <= qi).astype(fp16)
    m2 = np.stack([np.tile(mlow, (1, 4)), np.tile(mcau, (1, 4))])  # [2,128,512]

    ident = np.eye(128, dtype=fp16)
    ones2d = np.ones((128, 128), fp16)

    blk0 = K0 // BS
    in_maps = []
    for m in range(M):
        # fold softmax scale into wq so exp needs no scale parameter
        wq_m = np.asarray(wq[:, 512 * m:512 * (m + 1)], f32) * SCALE
        wk_m = np.asarray(wk[:, 128 * m:128 * (m + 1)], f32)
        wv_m = np.asarray(wv[:, 128 * m:128 * (m + 1)], f32)
        # feature blocks in pass order: (k,v), (q0,q1), (q2,q3)
        fblocks = [wk_m, wv_m, wq_m[:, 0:128], wq_m[:, 128:256],
                   wq_m[:, 256:384], wq_m[:, 384:512]]
        wcat = np.empty((3, 16, 128, 512), fp16)
        for p in range(3):
            a = fblocks[2 * p].reshape(32, 128, 128)
            bb = fblocks[2 * p + 1].reshape(32, 128, 128)
            pair = np.concatenate([a, bb], axis=2)      # [32, 128, 256]
            wcat[p] = pair.reshape(16, 2, 128, 256) \
                .transpose(0, 2, 1, 3).reshape(16, 128, 512)
        wo_m = np.asarray(wo[:, 512 * m:512 * (m + 1)], f32).astype(fp16)

        kcT_m = np.empty((B, 128, CKEYS), fp16)
        vc_m = np.empty((B, 128, CKEYS), fp16)
        for b in range(B):
            blks = block_offsets[b, blk0:blk0 + NCBLK]
            kc = np.asarray(k_cache[blks, :, m, :], f32)     # [16, 64, 128]
            vcb = np.asarray(v_cache[blks, :, m, :], f32)
            kcT_m[b] = kc.reshape(CKEYS, 128).T              # [128 d, keys]
            vc_m[b] = vcb.reshape(8, 128, 128).transpose(1, 0, 2).reshape(128, CKEYS)
        in_maps.append(dict(
            hidT=hidT, wcat=wcat, wo=wo_m,
            kcT=np.ascontiguousarray(kcT_m), vc=np.ascontiguousarray(vc_m),
            cosT=cosT, sinTs=sinTs, masks=m2, ident=ident, ones2d=ones2d))
    return in_maps


def kernel(**inputs):
    in_maps = _prep_inputs(**inputs)
    if "nc" not in _CACHE:
        _CACHE["nc"] = _build()
    nc = _CACHE["nc"]

    kwargs = {}
    if os.environ.get("KERNEL_TRACE"):
        import types as _types
        from trn_agent_boot.trn_boot import _ntff_profile_via_ctypes
        hook = _ntff_profile_via_ctypes('/opt/axon/libaxon_pjrt.so')
        mod = _types.ModuleType("antenv.axon_hooks")
        mod.get_axon_ntff_profile_hook = lambda: hook
        sys.modules["antenv.axon_hooks"] = mod
        tdir = os.environ.get("KERNEL_TRACE_DIR", "/tmp/kernel_trace")
        os.makedirs(tdir, exist_ok=True)
        kwargs = dict(trace=True, tmpdir=tdir)

    res = run_bass_kernel_spmd(nc, in_maps, core_ids=list(range(M)), **kwargs)
    if res.exec_time_ns is not None:
        print(f"HW exec time: {res.exec_time_ns} ns")
    out = np.concatenate([res.results[m]["out"] for m in range(M)], axis=1)
    return np.ascontiguousarray(out, np.float32)
